# revision 22
# baseline (speedup 1.0000x reference)
"""MHA kernel for Trainium2, 8-core tensor-parallel (2 heads per core).

Problem (hardcoded): x [2, 2048, 1024] fp32, Wq/Wk/Wv/Wo [1024, 1024],
bq/bk/bv/bo [1024], H=16 heads, DH=64.  out = MHA(x).

Sharding: heads split 8 ways (2 heads = 128 proj columns per core). Each
core computes its heads' attention and a row-parallel partial of the
output projection (written bf16); the host sums the 8 partials and adds
the closed-form bias terms (bv @ Wo + bo).

Pipeline design (all engines near-balanced, PE gap-free to keep the HAM
clock gate at 2.4 GHz):
  - P0: Q^T/K^T projection for batch 0, chunk-paced by the x^T DMA.
  - P1: scores(combo 0) interleaved with QK batch 1 + all V projection
    (token-major V with a ones column for the softmax denominator).
  - blocks c0..c7: per key-tile step: score-pair(ci+1) [two K=64
    matmuls on PE row-groups 0-1/2-3], two AV matmuls (ci), outproj
    matmuls (ci-1) spread through the block.
  - softmax exp split across engines: ACT does true Exp for 10/16 key
    tiles; DVE does a Schraudolph approx-exp for 6/16 (affine in fp32,
    convert to int16 == the bf16 bit pattern of exp(x), written straight
    into the P tile via a bitcast view).
  - normalization: denominator from the V ones-column; reciprocal on
    DVE (approx, ~51 ulp), partition-broadcast on GpSimd, multiply on
    DVE while converting to bf16.
  - outproj PSUM drained by ACT (activation Copy) and DVE alternately,
    DMA'd out as bf16.
"""

import numpy as np
import ml_dtypes

D = 1024
T = 4096          # B*S tokens
S = 2048
B = 2
NH = 2            # heads per core
DH = 64
NCORES = 8
NKT = 16          # 128-key tiles per batch
NQC = 4           # 512-query chunks per batch
# V slot layout (128 wide): col 0 = ones (softmax denominator), cols
# 64..127 = V columns.  AV psum then has denom at partition 0 (where the
# custom-DVE reciprocal works) and O at partitions 64..127 (32-aligned).
VSLOT = 128
SCALE = 0.125     # 1/sqrt(DH)

SCHR_KTS = (0, 3, 6, 8, 11, 14)  # key tiles per combo on DVE Schraudolph exp
C_MAGIC = 486411  # Schraudolph bias correction (min end-to-end error)
A16 = float(np.float32(2 ** 23 / np.log(2) * SCALE / 65536.0))
B16 = float(np.float32((127 * 2 ** 23 - C_MAGIC) / 65536.0 + 0.5))

_CACHE = {}


def _build_nc():
    import concourse.bacc as bacc
    import concourse.mybir as mybir
    import concourse.tile as tile

    dt = mybir.dt
    f32, bf16, i16 = dt.float32, dt.bfloat16, dt.int16
    Exp = mybir.ActivationFunctionType.Exp
    Copy = mybir.ActivationFunctionType.Copy
    Mult = mybir.AluOpType.mult
    Add = mybir.AluOpType.add

    nc = bacc.Bacc("TRN2", target_bir_lowering=False, debug=False,
                   num_devices=NCORES)

    xT = nc.dram_tensor("xT", [D, T], bf16, kind="ExternalInput")
    # weights pre-swizzled on host to the SBUF layout (partition-major)
    wq_d = nc.dram_tensor("wq", [128, D], bf16, kind="ExternalInput")
    wk_d = nc.dram_tensor("wk", [128, D], bf16, kind="ExternalInput")
    wv_d = nc.dram_tensor("wv", [128, D], bf16, kind="ExternalInput")
    wo_d = nc.dram_tensor("wo", [128, D], bf16, kind="ExternalInput")
    bq_d = nc.dram_tensor("bq", [128, 1], f32, kind="ExternalInput")
    bk_d = nc.dram_tensor("bk", [128, 1], f32, kind="ExternalInput")
    outp = nc.dram_tensor("outp", [T, D], bf16, kind="ExternalOutput")

    with tile.TileContext(nc) as tc:
        with (
            tc.tile_pool(name="persist", bufs=1) as pp,
            tc.tile_pool(name="pt", bufs=2) as ptp,
            tc.tile_pool(name="onorm", bufs=2) as onp,
            tc.tile_pool(name="rbp", bufs=2) as rbp,
            tc.tile_pool(name="rcp", bufs=2) as rcp,
            tc.tile_pool(name="outsb", bufs=4) as osp,
            tc.tile_pool(name="ps", bufs=1, space="PSUM") as psp,
        ):
            # ---- weight / bias / x DMAs (SP queue, in emission order) ----
            wq = pp.tile([128, D], bf16, tag="wq")
            wk = pp.tile([128, D], bf16, tag="wk")
            wv = pp.tile([128, D], bf16, tag="wv")
            wo = pp.tile([128, D], bf16, tag="wo")
            bq = pp.tile([128, 1], f32, tag="bq")
            bk = pp.tile([128, 1], f32, tag="bk")
            for w_sb, w_dr in ((wq, wq_d), (wk, wk_d)):
                nc.sync.dma_start(out=w_sb[:, :], in_=w_dr.ap()[:, :])
            nc.sync.dma_start(out=bq[:, :], in_=bq_d.ap()[:, :])
            nc.sync.dma_start(out=bk[:, :], in_=bk_d.ap()[:, :])

            xta = pp.tile([128, 8 * T], bf16, tag="xta")
            xt4 = xta.rearrange("p (d t) -> p d t", d=8)
            xt = [xt4[:, d, :] for d in range(8)]
            xT_src = xT.ap().rearrange("(d p) t -> p d t", p=128)

            def load_x_chunk(c):
                # one dma_start per d-tile so the descriptors spread across
                # the DMA rings
                cs = slice(c * 512, (c + 1) * 512)
                for d in range(8):
                    nc.sync.dma_start(out=xt4[:, d, cs],
                                      in_=xT_src[:, d, cs])

            for c in range(5):
                load_x_chunk(c)
            nc.sync.dma_start(out=wv[:, :], in_=wv_d.ap()[:, :])
            nc.sync.dma_start(out=wo[:, :], in_=wo_d.ap()[:, :])
            for c in range(5, 8):
                load_x_chunk(c)

            wq3 = wq.rearrange("p (t c) -> p t c", c=128)
            wk3 = wk.rearrange("p (t c) -> p t c", c=128)
            wv3 = wv.rearrange("p (t c) -> p t c", c=128)

            # ---- persistent SBUF state ----
            qt = pp.tile([128, T], bf16, tag="qt")
            kt = pp.tile([128, T], bf16, tag="kt")
            vtm = []
            for b in range(B):
                v_sb = pp.tile([128, NH * NKT * VSLOT], bf16, tag=f"v{b}",
                               name=f"v{b}")
                v4 = v_sb.rearrange("p (h k c) -> p h k c", h=NH, k=NKT)
                nc.vector.memset(v4[:, :, :, 0:DH], 0.0)
                nc.vector.memset(v4[:, :, :, 0:1], 1.0)
                vtm.append(v_sb)

            pt_tiles = {}

            def pt_for(ci):
                if ci not in pt_tiles:
                    t = ptp.tile([128, NH * NKT * 512], bf16, tag="pt",
                                 name=f"pt{ci}")
                    pt_tiles[ci] = t.rearrange("p (h k q) -> p h k q",
                                               h=NH, k=NKT)
                return pt_tiles[ci]

            # ---- emission helpers ----
            drain_flip = [0]

            def emit_qk_group(proj_sb, w3, b_sb, c):
                cs = slice(c * 512, (c + 1) * 512)
                ps = psp.tile([128, 512], f32, tag="acc", bufs=2, name="qkps")
                for d in range(8):
                    nc.tensor.matmul(ps[:, :], w3[:, d, :], xt[d][:, cs],
                                     start=(d == 0), stop=(d == 7))
                nc.vector.tensor_scalar_add(proj_sb[:, cs], ps[:, :],
                                            b_sb[:, :])

            def emit_v_group(b, k, slot):
                tok0 = b * S + k * 128
                ps = psp.tile([128, 128], f32, tag=slot,
                              bufs=(2 if slot == "acc" else 1),
                              padded_shape=([128, 1024] if slot == "op"
                                            else [128, 512]),
                              name=f"vps{b}_{k}")
                for d in range(8):
                    nc.tensor.matmul(ps[:, :], xt[d][:, tok0:tok0 + 128],
                                     wv3[:, d, :],
                                     start=(d == 0), stop=(d == 7))
                v4 = vtm[b].rearrange("p (h k c) -> p h k c", h=NH, k=NKT)
                dst = v4[:, :, k, DH:2 * DH]
                src = ps.rearrange("p (h c) -> p h c", h=NH)[:, :, :]
                if drain_flip[0] % 2 == 0:
                    nc.vector.tensor_copy(dst, src)
                else:
                    nc.scalar.activation(dst, src, Copy)
                drain_flip[0] += 1

            def emit_spair(ci, k):
                b, qc = divmod(ci, NQC)
                q0 = b * S + qc * 512
                k0 = b * S + k * 128
                st = psp.tile([128, 1024], f32, tag="st", bufs=2,
                              name=f"st{ci}_{k}")
                for h in range(NH):
                    hp = h * DH
                    nc.tensor.matmul(
                        st[:, h * 512:(h + 1) * 512],
                        kt[hp:hp + DH, k0:k0 + 128],
                        qt[hp:hp + DH, q0:q0 + 512],
                        start=True, stop=True)
                p4 = pt_for(ci)
                if k in SCHR_KTS:
                    out_i16 = p4[:, :, k, :].bitcast(i16)
                    nc.vector.tensor_scalar(
                        out=out_i16, in0=st[:, :], scalar1=A16, scalar2=B16,
                        op0=Mult, op1=Add)
                else:
                    nc.scalar.activation(p4[:, :, k, :], st[:, :], Exp,
                                         scale=SCALE)

            av_tiles = {}

            def emit_av(ci, h, k):
                b = ci // NQC
                v4 = vtm[b].rearrange("p (h k c) -> p h k c", h=NH, k=NKT)
                if (ci, h) not in av_tiles:
                    av_tiles[(ci, h)] = psp.tile(
                        [VSLOT, 512], f32, tag="acc", bufs=2,
                        padded_shape=[128, 512], name=f"av{ci}_{h}")
                av = av_tiles[(ci, h)]
                nc.tensor.matmul(av[0:128, :], v4[:, h, k, :],
                                 pt_for(ci)[:, h, k, :],
                                 start=(k == 0), stop=(k == NKT - 1))

            onorm_tiles = {}

            def emit_norm(ci, h):
                # denom -> reciprocal (DVE) -> broadcast (GpSimd) ->
                # multiply+bf16 (DVE)
                av = av_tiles[(ci, h)]
                recip = rcp.tile([1, 512], f32, tag="recip",
                                 name=f"rc{ci}_{h}")
                nc.vector.reciprocal_approx_fast(out=recip[:, :],
                                                 in_=av[0:1, :])
                rb = rbp.tile([DH, 512], f32, tag="rb", name=f"rb{ci}_{h}")
                nc.gpsimd.partition_broadcast(rb[:, :], recip[:, :])
                if ci not in onorm_tiles:
                    onorm_tiles[ci] = onp.tile([128, 512], bf16, tag="onorm",
                                               name=f"on{ci}")
                onorm = onorm_tiles[ci]
                hp = h * DH
                nc.vector.tensor_tensor(onorm[hp:hp + DH, :],
                                        av[DH:2 * DH, :],
                                        rb[:, :], op=Mult)

            op_tiles = {}

            def _drain_op(ci, s4):
                op = op_tiles[(ci, s4)]
                osb = osp.tile([128, 1024], bf16, tag="outsb",
                               name=f"osb{ci}_{s4}")
                if s4 % 2 == 0:
                    nc.scalar.activation(osb[:, :], op[:, :], Copy)
                else:
                    nc.vector.tensor_copy(osb[:, :], op[:, :])
                b, qc = divmod(ci, NQC)
                r0 = b * S + qc * 512 + s4 * 128
                nc.sync.dma_start(out=outp.ap()[r0:r0 + 128, :],
                                  in_=osb[:, :])

            def emit_outproj_mm(ci, j):
                # j = 0..7 -> s4 = j//2 (128-token block), jc = j%2
                s4, jc = divmod(j, 2)
                onorm = onorm_tiles[ci]
                if (ci, s4) not in op_tiles:
                    op_tiles[(ci, s4)] = psp.tile(
                        [128, 1024], f32, tag="op", bufs=1,
                        name=f"op{ci}_{s4}")
                op = op_tiles[(ci, s4)]
                nc.tensor.matmul(op[:, jc * 512:(jc + 1) * 512],
                                 onorm[:, s4 * 128:(s4 + 1) * 128],
                                 wo[:, jc * 512:(jc + 1) * 512],
                                 start=True, stop=True)
                if jc == 1:
                    _drain_op(ci, s4)

            def emit_outproj_half(ci, j, h):
                # per-head K=64 outproj half (tail only): the h0 half can run
                # before the h1 norm chain completes
                s4, jc = divmod(j, 2)
                onorm = onorm_tiles[ci]
                if (ci, s4) not in op_tiles:
                    # borrow the (now idle) st slots: two tiles in flight
                    op_tiles[(ci, s4)] = psp.tile(
                        [128, 1024], f32, tag="st", bufs=2,
                        name=f"op{ci}_{s4}")
                op = op_tiles[(ci, s4)]
                hp = h * DH
                nc.tensor.matmul(op[:, jc * 512:(jc + 1) * 512],
                                 onorm[hp:hp + DH, s4 * 128:(s4 + 1) * 128],
                                 wo[hp:hp + DH, jc * 512:(jc + 1) * 512],
                                 start=(h == 0), stop=(h == 1))

            # ================= P0: QK projection, batch 0 =================
            for c in range(NQC):
                emit_qk_group(qt, wq3, bq, c)
                emit_qk_group(kt, wk3, bk, c)

            # ===== P1: scores(c0) + QK batch 1 + V both batches =====
            for k in range(NKT):
                emit_spair(0, k)
                if k < 8:
                    c = 4 + k // 2
                    if k % 2 == 0:
                        emit_qk_group(qt, wq3, bq, c)
                    else:
                        emit_qk_group(kt, wk3, bk, c)
                g = 2 * k
                emit_v_group(g // NKT, g % NKT, "acc")
                g = 2 * k + 1
                emit_v_group(g // NKT, g % NKT, "op")

            # ================= attention blocks =================
            # per step: AVs and outproj first, score-pair last — gives the
            # exp engines maximal slack before the PE needs the st slot back
            for ci in range(8):
                for k in range(NKT):
                    h, kk = k // 8, 2 * (k % 8)
                    if k == 8:
                        emit_norm(ci, 0)
                    emit_av(ci, h, kk)
                    if ci + 1 < 8:
                        emit_spair(ci + 1, k)
                    emit_av(ci, h, kk + 1)
                    if ci >= 1 and 4 <= k < 12:
                        emit_outproj_mm(ci - 1, k - 4)
                emit_norm(ci, 1)

            # tail: outproj of the last combo on the idle st slots
            for s4 in range(4):
                for jc in range(2):
                    onorm = onorm_tiles[7]
                    if (7, s4) not in op_tiles:
                        op_tiles[(7, s4)] = psp.tile(
                            [128, 1024], f32, tag="st", bufs=2,
                            name=f"op7_{s4}")
                    op = op_tiles[(7, s4)]
                    nc.tensor.matmul(op[:, jc * 512:(jc + 1) * 512],
                                     onorm[:, s4 * 128:(s4 + 1) * 128],
                                     wo[:, jc * 512:(jc + 1) * 512],
                                     start=True, stop=True)
                _drain_op(7, s4)

    nc.compile()
    return nc


def _prep_inputs(x, Wq, bq, Wk, bk, Wv, bv, Wo, bo):
    bf16 = ml_dtypes.bfloat16
    xT = np.ascontiguousarray(
        np.asarray(x, dtype=np.float32).reshape(T, D).T).astype(bf16)

    def swz(W, cs):
        # [D, 128] -> SBUF layout [128 p, 8 t, 128 c] flattened
        return np.ascontiguousarray(
            W[:, cs].reshape(8, 128, 128).transpose(1, 0, 2).reshape(128, D)
        ).astype(bf16)

    in_maps = []
    for c in range(NCORES):
        cs = slice(c * 128, (c + 1) * 128)
        in_maps.append({
            "xT": xT,
            "wq": swz(Wq, cs),
            "wk": swz(Wk, cs),
            "wv": swz(Wv, cs),
            "wo": np.ascontiguousarray(Wo[cs, :]).astype(bf16),
            "bq": np.ascontiguousarray(bq[cs]).reshape(128, 1).astype(np.float32),
            "bk": np.ascontiguousarray(bk[cs]).reshape(128, 1).astype(np.float32),
        })
    return in_maps


def kernel(x, Wq, bq, Wk, bk, Wv, bv, Wo, bo, _trace=False, _results=None):
    from concourse.bass_utils import run_bass_kernel_spmd

    x = np.asarray(x); Wq = np.asarray(Wq); Wk = np.asarray(Wk)
    Wv = np.asarray(Wv); Wo = np.asarray(Wo)
    bq = np.asarray(bq); bk = np.asarray(bk); bv = np.asarray(bv)
    bo = np.asarray(bo)

    if "nc" not in _CACHE:
        _CACHE["nc"] = _build_nc()
    nc = _CACHE["nc"]

    in_maps = _prep_inputs(x, Wq, bq, Wk, bk, Wv, bv, Wo, bo)
    res = run_bass_kernel_spmd(
        nc, in_maps, core_ids=list(range(NCORES)), trace=_trace)
    if _results is not None:
        _results.append(res)

    acc = np.zeros((T, D), dtype=np.float32)
    for c in range(NCORES):
        acc += np.asarray(res.results[c]["outp"], dtype=np.float32)
    acc += bv.astype(np.float32) @ Wo.astype(np.float32) + bo.astype(np.float32)
    return acc.reshape(B, S, D)


# revision 24
# speedup vs baseline: 1.0173x; 1.0173x over previous
"""MHA kernel for Trainium2, 8-core tensor-parallel (2 heads per core).

Problem (hardcoded): x [2, 2048, 1024] fp32, Wq/Wk/Wv/Wo [1024, 1024],
bq/bk/bv/bo [1024], H=16 heads, DH=64.  out = MHA(x).

Sharding: heads split 8 ways (2 heads = 128 proj columns per core). Each
core computes its heads' attention and a row-parallel partial of the
output projection (written bf16); the host sums the 8 partials and adds
the closed-form bias terms (bv @ Wo + bo).

Pipeline design (all engines near-balanced, PE gap-free to keep the HAM
clock gate at 2.4 GHz):
  - P0: Q^T/K^T projection for batch 0, chunk-paced by the x^T DMA.
  - P1: scores(combo 0) interleaved with QK batch 1 + all V projection
    (token-major V with a ones column for the softmax denominator).
  - blocks c0..c7: per key-tile step: score-pair(ci+1) [two K=64
    matmuls on PE row-groups 0-1/2-3], two AV matmuls (ci), outproj
    matmuls (ci-1) spread through the block.
  - softmax exp split across engines: ACT does true Exp for 10/16 key
    tiles; DVE does a Schraudolph approx-exp for 6/16 (affine in fp32,
    convert to int16 == the bf16 bit pattern of exp(x), written straight
    into the P tile via a bitcast view).
  - normalization: denominator from the V ones-column; reciprocal on
    DVE (approx, ~51 ulp), partition-broadcast on GpSimd, multiply on
    DVE while converting to bf16.
  - outproj PSUM drained by ACT (activation Copy) and DVE alternately,
    DMA'd out as bf16.
"""

import numpy as np
import ml_dtypes

D = 1024
T = 4096          # B*S tokens
S = 2048
B = 2
NH = 2            # heads per core
DH = 64
NCORES = 8
NKT = 16          # 128-key tiles per batch
NQC = 4           # 512-query chunks per batch
# V slot layout (128 wide): col 0 = ones (softmax denominator), cols
# 64..127 = V columns.  AV psum then has denom at partition 0 (where the
# custom-DVE reciprocal works) and O at partitions 64..127 (32-aligned).
VSLOT = 128
SCALE = 0.125     # 1/sqrt(DH)

SCHR_KTS = (0, 3, 6, 8, 11, 14)  # key tiles per combo on DVE Schraudolph exp
C_MAGIC = 486411  # Schraudolph bias correction (min end-to-end error)
A16 = float(np.float32(2 ** 23 / np.log(2) * SCALE / 65536.0))
B16 = float(np.float32((127 * 2 ** 23 - C_MAGIC) / 65536.0 + 0.5))

_CACHE = {}


def _build_nc():
    import concourse.bacc as bacc
    import concourse.mybir as mybir
    import concourse.tile as tile

    dt = mybir.dt
    f32, bf16, i16 = dt.float32, dt.bfloat16, dt.int16
    Exp = mybir.ActivationFunctionType.Exp
    Copy = mybir.ActivationFunctionType.Copy
    Mult = mybir.AluOpType.mult
    Add = mybir.AluOpType.add

    nc = bacc.Bacc("TRN2", target_bir_lowering=False, debug=False,
                   num_devices=NCORES)

    xT = nc.dram_tensor("xT", [D, T], bf16, kind="ExternalInput")
    # weights pre-swizzled on host to the SBUF layout (partition-major)
    wq_d = nc.dram_tensor("wq", [128, D], bf16, kind="ExternalInput")
    wk_d = nc.dram_tensor("wk", [128, D], bf16, kind="ExternalInput")
    wv_d = nc.dram_tensor("wv", [128, D], bf16, kind="ExternalInput")
    wo_d = nc.dram_tensor("wo", [128, D], bf16, kind="ExternalInput")
    bq_d = nc.dram_tensor("bq", [128, 1], f32, kind="ExternalInput")
    bk_d = nc.dram_tensor("bk", [128, 1], f32, kind="ExternalInput")
    outp = nc.dram_tensor("outp", [T, D], bf16, kind="ExternalOutput")

    with tile.TileContext(nc) as tc:
        with (
            tc.tile_pool(name="persist", bufs=1) as pp,
            tc.tile_pool(name="pt", bufs=2) as ptp,
            tc.tile_pool(name="onorm", bufs=2) as onp,
            tc.tile_pool(name="rbp", bufs=2) as rbp,
            tc.tile_pool(name="rcp", bufs=2) as rcp,
            tc.tile_pool(name="outsb", bufs=4) as osp,
            tc.tile_pool(name="ps", bufs=1, space="PSUM") as psp,
        ):
            # ---- weight / bias / x DMAs (SP queue, in emission order) ----
            wq = pp.tile([128, D], bf16, tag="wq")
            wk = pp.tile([128, D], bf16, tag="wk")
            wv = pp.tile([128, D], bf16, tag="wv")
            wo = pp.tile([128, D], bf16, tag="wo")
            bq = pp.tile([128, 1], f32, tag="bq")
            bk = pp.tile([128, 1], f32, tag="bk")
            for w_sb, w_dr in ((wq, wq_d), (wk, wk_d)):
                nc.sync.dma_start(out=w_sb[:, :], in_=w_dr.ap()[:, :])
            nc.sync.dma_start(out=bq[:, :], in_=bq_d.ap()[:, :])
            nc.sync.dma_start(out=bk[:, :], in_=bk_d.ap()[:, :])

            xta = pp.tile([128, 8 * T], bf16, tag="xta")
            xt4 = xta.rearrange("p (d t) -> p d t", d=8)
            xt = [xt4[:, d, :] for d in range(8)]
            xT_src = xT.ap().rearrange("(d p) t -> p d t", p=128)

            def load_x_chunk(c):
                cs = slice(c * 512, (c + 1) * 512)
                nc.sync.dma_start(out=xt4[:, :, cs], in_=xT_src[:, :, cs])

            for c in range(5):
                load_x_chunk(c)
            nc.sync.dma_start(out=wv[:, :], in_=wv_d.ap()[:, :])
            nc.sync.dma_start(out=wo[:, :], in_=wo_d.ap()[:, :])
            for c in range(5, 8):
                load_x_chunk(c)

            wq3 = wq.rearrange("p (t c) -> p t c", c=128)
            wk3 = wk.rearrange("p (t c) -> p t c", c=128)
            wv3 = wv.rearrange("p (t c) -> p t c", c=128)

            # ---- persistent SBUF state ----
            qt = pp.tile([128, T], bf16, tag="qt")
            kt = pp.tile([128, T], bf16, tag="kt")
            vtm = []
            for b in range(B):
                v_sb = pp.tile([128, NH * NKT * VSLOT], bf16, tag=f"v{b}",
                               name=f"v{b}")
                v4 = v_sb.rearrange("p (h k c) -> p h k c", h=NH, k=NKT)
                nc.vector.memset(v4[:, :, :, 0:DH], 0.0)
                nc.vector.memset(v4[:, :, :, 0:1], 1.0)
                vtm.append(v_sb)

            pt_tiles = {}

            def pt_for(ci):
                if ci not in pt_tiles:
                    t = ptp.tile([128, NH * NKT * 512], bf16, tag="pt",
                                 name=f"pt{ci}")
                    pt_tiles[ci] = t.rearrange("p (h k q) -> p h k q",
                                               h=NH, k=NKT)
                return pt_tiles[ci]

            # ---- emission helpers ----
            drain_flip = [0]

            def emit_qk_group(proj_sb, w3, b_sb, c):
                cs = slice(c * 512, (c + 1) * 512)
                ps = psp.tile([128, 512], f32, tag="acc", bufs=2, name="qkps")
                for d in range(8):
                    nc.tensor.matmul(ps[:, :], w3[:, d, :], xt[d][:, cs],
                                     start=(d == 0), stop=(d == 7))
                nc.vector.tensor_scalar_add(proj_sb[:, cs], ps[:, :],
                                            b_sb[:, :])

            def emit_v_group(b, k, slot):
                tok0 = b * S + k * 128
                ps = psp.tile([128, 128], f32, tag=slot,
                              bufs=(2 if slot == "acc" else 1),
                              padded_shape=([128, 1024] if slot == "op"
                                            else [128, 512]),
                              name=f"vps{b}_{k}")
                for d in range(8):
                    nc.tensor.matmul(ps[:, :], xt[d][:, tok0:tok0 + 128],
                                     wv3[:, d, :],
                                     start=(d == 0), stop=(d == 7))
                v4 = vtm[b].rearrange("p (h k c) -> p h k c", h=NH, k=NKT)
                dst = v4[:, :, k, DH:2 * DH]
                src = ps.rearrange("p (h c) -> p h c", h=NH)[:, :, :]
                if drain_flip[0] % 2 == 0:
                    nc.vector.tensor_copy(dst, src)
                else:
                    nc.scalar.activation(dst, src, Copy)
                drain_flip[0] += 1

            def emit_spair(ci, k):
                b, qc = divmod(ci, NQC)
                q0 = b * S + qc * 512
                k0 = b * S + k * 128
                st = psp.tile([128, 1024], f32, tag="st", bufs=2,
                              name=f"st{ci}_{k}")
                for h in range(NH):
                    hp = h * DH
                    nc.tensor.matmul(
                        st[:, h * 512:(h + 1) * 512],
                        kt[hp:hp + DH, k0:k0 + 128],
                        qt[hp:hp + DH, q0:q0 + 512],
                        start=True, stop=True)
                p4 = pt_for(ci)
                if k in SCHR_KTS:
                    out_i16 = p4[:, :, k, :].bitcast(i16)
                    nc.vector.tensor_scalar(
                        out=out_i16, in0=st[:, :], scalar1=A16, scalar2=B16,
                        op0=Mult, op1=Add)
                else:
                    nc.scalar.activation(p4[:, :, k, :], st[:, :], Exp,
                                         scale=SCALE)

            av_tiles = {}

            def emit_av(ci, h, k):
                b = ci // NQC
                v4 = vtm[b].rearrange("p (h k c) -> p h k c", h=NH, k=NKT)
                if (ci, h) not in av_tiles:
                    av_tiles[(ci, h)] = psp.tile(
                        [VSLOT, 512], f32, tag="acc", bufs=2,
                        padded_shape=[128, 512], name=f"av{ci}_{h}")
                av = av_tiles[(ci, h)]
                nc.tensor.matmul(av[0:128, :], v4[:, h, k, :],
                                 pt_for(ci)[:, h, k, :],
                                 start=(k == 0), stop=(k == NKT - 1))

            onorm_tiles = {}

            def emit_norm(ci, h):
                # denom -> reciprocal (DVE) -> broadcast (GpSimd) ->
                # multiply+bf16 (DVE)
                av = av_tiles[(ci, h)]
                recip = rcp.tile([1, 512], f32, tag="recip",
                                 name=f"rc{ci}_{h}")
                nc.vector.reciprocal_approx_fast(out=recip[:, :],
                                                 in_=av[0:1, :])
                rb = rbp.tile([DH, 512], f32, tag="rb", name=f"rb{ci}_{h}")
                nc.gpsimd.partition_broadcast(rb[:, :], recip[:, :])
                if ci not in onorm_tiles:
                    onorm_tiles[ci] = onp.tile([128, 512], bf16, tag="onorm",
                                               name=f"on{ci}")
                onorm = onorm_tiles[ci]
                hp = h * DH
                nc.vector.tensor_tensor(onorm[hp:hp + DH, :],
                                        av[DH:2 * DH, :],
                                        rb[:, :], op=Mult)

            op_tiles = {}

            def _drain_op(ci, s4):
                op = op_tiles[(ci, s4)]
                osb = osp.tile([128, 1024], bf16, tag="outsb",
                               name=f"osb{ci}_{s4}")
                if s4 % 2 == 0:
                    nc.scalar.activation(osb[:, :], op[:, :], Copy)
                else:
                    nc.vector.tensor_copy(osb[:, :], op[:, :])
                b, qc = divmod(ci, NQC)
                r0 = b * S + qc * 512 + s4 * 128
                nc.sync.dma_start(out=outp.ap()[r0:r0 + 128, :],
                                  in_=osb[:, :])

            def emit_outproj_mm(ci, j):
                # j = 0..7 -> s4 = j//2 (128-token block), jc = j%2
                s4, jc = divmod(j, 2)
                onorm = onorm_tiles[ci]
                if (ci, s4) not in op_tiles:
                    op_tiles[(ci, s4)] = psp.tile(
                        [128, 1024], f32, tag="op", bufs=1,
                        name=f"op{ci}_{s4}")
                op = op_tiles[(ci, s4)]
                nc.tensor.matmul(op[:, jc * 512:(jc + 1) * 512],
                                 onorm[:, s4 * 128:(s4 + 1) * 128],
                                 wo[:, jc * 512:(jc + 1) * 512],
                                 start=True, stop=True)
                if jc == 1:
                    _drain_op(ci, s4)

            def emit_outproj_half(ci, j, h):
                # per-head K=64 outproj half (tail only): the h0 half can run
                # before the h1 norm chain completes
                s4, jc = divmod(j, 2)
                onorm = onorm_tiles[ci]
                if (ci, s4) not in op_tiles:
                    # borrow the (now idle) st slots: two tiles in flight
                    op_tiles[(ci, s4)] = psp.tile(
                        [128, 1024], f32, tag="st", bufs=2,
                        name=f"op{ci}_{s4}")
                op = op_tiles[(ci, s4)]
                hp = h * DH
                nc.tensor.matmul(op[:, jc * 512:(jc + 1) * 512],
                                 onorm[hp:hp + DH, s4 * 128:(s4 + 1) * 128],
                                 wo[hp:hp + DH, jc * 512:(jc + 1) * 512],
                                 start=(h == 0), stop=(h == 1))

            # ================= P0: QK projection, batch 0 =================
            for c in range(NQC):
                emit_qk_group(qt, wq3, bq, c)
                emit_qk_group(kt, wk3, bk, c)

            # ===== P1: scores(c0) + QK batch 1 + V both batches =====
            for k in range(NKT):
                emit_spair(0, k)
                if k < 8:
                    c = 4 + k // 2
                    if k % 2 == 0:
                        emit_qk_group(qt, wq3, bq, c)
                    else:
                        emit_qk_group(kt, wk3, bk, c)
                g = 2 * k
                emit_v_group(g // NKT, g % NKT, "acc")
                g = 2 * k + 1
                emit_v_group(g // NKT, g % NKT, "op")

            # ================= attention blocks =================
            # per step: AVs and outproj first, score-pair last — gives the
            # exp engines maximal slack before the PE needs the st slot back
            for ci in range(8):
                for k in range(NKT):
                    h, kk = k // 8, 2 * (k % 8)
                    if k == 8:
                        emit_norm(ci, 0)
                    emit_av(ci, h, kk)
                    emit_av(ci, h, kk + 1)
                    if ci >= 1 and 4 <= k < 12:
                        emit_outproj_mm(ci - 1, k - 4)
                    if ci + 1 < 8:
                        emit_spair(ci + 1, k)
                emit_norm(ci, 1)

            # tail: outproj of the last combo on the idle st slots
            for s4 in range(4):
                for jc in range(2):
                    onorm = onorm_tiles[7]
                    if (7, s4) not in op_tiles:
                        op_tiles[(7, s4)] = psp.tile(
                            [128, 1024], f32, tag="st", bufs=2,
                            name=f"op7_{s4}")
                    op = op_tiles[(7, s4)]
                    nc.tensor.matmul(op[:, jc * 512:(jc + 1) * 512],
                                     onorm[:, s4 * 128:(s4 + 1) * 128],
                                     wo[:, jc * 512:(jc + 1) * 512],
                                     start=True, stop=True)
                _drain_op(7, s4)

    nc.compile()
    return nc


def _prep_inputs(x, Wq, bq, Wk, bk, Wv, bv, Wo, bo):
    bf16 = ml_dtypes.bfloat16
    xT = np.ascontiguousarray(
        np.asarray(x, dtype=np.float32).reshape(T, D).T).astype(bf16)

    def swz(W, cs):
        # [D, 128] -> SBUF layout [128 p, 8 t, 128 c] flattened
        return np.ascontiguousarray(
            W[:, cs].reshape(8, 128, 128).transpose(1, 0, 2).reshape(128, D)
        ).astype(bf16)

    in_maps = []
    for c in range(NCORES):
        cs = slice(c * 128, (c + 1) * 128)
        in_maps.append({
            "xT": xT,
            "wq": swz(Wq, cs),
            "wk": swz(Wk, cs),
            "wv": swz(Wv, cs),
            "wo": np.ascontiguousarray(Wo[cs, :]).astype(bf16),
            "bq": np.ascontiguousarray(bq[cs]).reshape(128, 1).astype(np.float32),
            "bk": np.ascontiguousarray(bk[cs]).reshape(128, 1).astype(np.float32),
        })
    return in_maps


def kernel(x, Wq, bq, Wk, bk, Wv, bv, Wo, bo, _trace=False, _results=None):
    from concourse.bass_utils import run_bass_kernel_spmd

    x = np.asarray(x); Wq = np.asarray(Wq); Wk = np.asarray(Wk)
    Wv = np.asarray(Wv); Wo = np.asarray(Wo)
    bq = np.asarray(bq); bk = np.asarray(bk); bv = np.asarray(bv)
    bo = np.asarray(bo)

    if "nc" not in _CACHE:
        _CACHE["nc"] = _build_nc()
    nc = _CACHE["nc"]

    in_maps = _prep_inputs(x, Wq, bq, Wk, bk, Wv, bv, Wo, bo)
    res = run_bass_kernel_spmd(
        nc, in_maps, core_ids=list(range(NCORES)), trace=_trace)
    if _results is not None:
        _results.append(res)

    acc = np.zeros((T, D), dtype=np.float32)
    for c in range(NCORES):
        acc += np.asarray(res.results[c]["outp"], dtype=np.float32)
    acc += bv.astype(np.float32) @ Wo.astype(np.float32) + bo.astype(np.float32)
    return acc.reshape(B, S, D)


# revision 27
# speedup vs baseline: 1.0336x; 1.0159x over previous
"""MHA kernel for Trainium2, 8-core tensor-parallel (2 heads per core).

Problem (hardcoded): x [2, 2048, 1024] fp32, Wq/Wk/Wv/Wo [1024, 1024],
bq/bk/bv/bo [1024], H=16 heads, DH=64.  out = MHA(x).

Sharding: heads split 8 ways (2 heads = 128 proj columns per core). Each
core computes its heads' attention and a row-parallel partial of the
output projection (written bf16); the host sums the 8 partials and adds
the closed-form bias terms (bv @ Wo + bo).

Pipeline design (all engines near-balanced, PE kept dense so the HAM
clock gate stays at 2.4 GHz; measured 197.4 us vs 279.5 us baseline):
  - P0: Q^T/K^T projection for batch 0, chunk-paced by the x^T DMA
    (weights pre-swizzled on the host so every DMA is 2KB descriptors).
  - P1: scores(combo 0) interleaved with QK batch 1 + all V projection
    (token-major V slots [ones | pad | V] so the AV psum has the softmax
    denominator at partition 0 and O at partitions 64..127).
  - blocks c0..c7, per key-tile step: two AV matmuls (ci), an outproj
    matmul (ci-1, spread over the block), then the score-pair (ci+1)
    [two K=64 matmuls on PE row-groups 0-1/2-3, auto tile_position].
  - softmax exp split across engines: ACT does true Exp for 10/16 key
    tiles; DVE does a Schraudolph approx-exp for 6/16 in ONE op (affine
    in fp32, convert to int16 == the bf16 bit pattern of exp(x), written
    into the P tile via a bitcast view; rms err ~1.8%, which the
    self-consistent denominator mostly cancels).
  - normalization: reciprocal_approx_fast on DVE (input must sit at
    PSUM partition 0 — it returns garbage at partition offset 64),
    partition-broadcast on GpSimd, multiply+bf16 on DVE.
  - outproj PSUM [128,1024] drained by ACT (Copy) / DVE alternately,
    DMA'd out as bf16 partials; host sums in fp32 and adds the
    closed-form bias terms.
PSUM budget (exactly 8 banks): st 2x[128,1024] + acc 2x[128,512]
(QK/V/AV accumulators) + op 1x[128,1024]; the tail combo's outproj
borrows the idle st slots.
"""

import numpy as np
import ml_dtypes

D = 1024
T = 4096          # B*S tokens
S = 2048
B = 2
NH = 2            # heads per core
DH = 64
NCORES = 8
NKT = 16          # 128-key tiles per batch
NQC = 4           # 512-query chunks per batch
# V slot layout (128 wide): col 0 = ones (softmax denominator), cols
# 64..127 = V columns.  AV psum then has denom at partition 0 (where the
# custom-DVE reciprocal works) and O at partitions 64..127 (32-aligned).
VSLOT = 128
SCALE = 0.125     # 1/sqrt(DH)

SCHR_KTS = (0, 3, 6, 8, 11, 14)  # key tiles per combo on DVE Schraudolph exp
C_MAGIC = 486411  # Schraudolph bias correction (min end-to-end error)
A16 = float(np.float32(2 ** 23 / np.log(2) * SCALE / 65536.0))
B16 = float(np.float32((127 * 2 ** 23 - C_MAGIC) / 65536.0 + 0.5))

_CACHE = {}


def _build_nc():
    import concourse.bacc as bacc
    import concourse.mybir as mybir
    import concourse.tile as tile

    dt = mybir.dt
    f32, bf16, i16 = dt.float32, dt.bfloat16, dt.int16
    Exp = mybir.ActivationFunctionType.Exp
    Copy = mybir.ActivationFunctionType.Copy
    Mult = mybir.AluOpType.mult
    Add = mybir.AluOpType.add

    nc = bacc.Bacc("TRN2", target_bir_lowering=False, debug=False,
                   num_devices=NCORES)

    xT = nc.dram_tensor("xT", [D, T], bf16, kind="ExternalInput")
    # weights pre-swizzled on host to the SBUF layout (partition-major)
    wq_d = nc.dram_tensor("wq", [128, D], bf16, kind="ExternalInput")
    wk_d = nc.dram_tensor("wk", [128, D], bf16, kind="ExternalInput")
    wv_d = nc.dram_tensor("wv", [128, D], bf16, kind="ExternalInput")
    wo_d = nc.dram_tensor("wo", [128, D], bf16, kind="ExternalInput")
    bq_d = nc.dram_tensor("bq", [128, 1], f32, kind="ExternalInput")
    bk_d = nc.dram_tensor("bk", [128, 1], f32, kind="ExternalInput")
    outp = nc.dram_tensor("outp", [T, D], bf16, kind="ExternalOutput")

    with tile.TileContext(nc) as tc:
        with (
            tc.tile_pool(name="persist", bufs=1) as pp,
            tc.tile_pool(name="pt", bufs=2) as ptp,
            tc.tile_pool(name="onorm", bufs=2) as onp,
            tc.tile_pool(name="rbp", bufs=2) as rbp,
            tc.tile_pool(name="rcp", bufs=2) as rcp,
            tc.tile_pool(name="outsb", bufs=4) as osp,
            tc.tile_pool(name="ps", bufs=1, space="PSUM") as psp,
        ):
            # ---- weight / bias / x DMAs (SP queue, in emission order) ----
            wq = pp.tile([128, D], bf16, tag="wq")
            wk = pp.tile([128, D], bf16, tag="wk")
            wv = pp.tile([128, D], bf16, tag="wv")
            wo = pp.tile([128, D], bf16, tag="wo")
            bq = pp.tile([128, 1], f32, tag="bq")
            bk = pp.tile([128, 1], f32, tag="bk")
            for w_sb, w_dr in ((wq, wq_d), (wk, wk_d)):
                nc.sync.dma_start(out=w_sb[:, :], in_=w_dr.ap()[:, :])
            nc.sync.dma_start(out=bq[:, :], in_=bq_d.ap()[:, :])
            nc.sync.dma_start(out=bk[:, :], in_=bk_d.ap()[:, :])

            xta = pp.tile([128, 8 * T], bf16, tag="xta")
            xt4 = xta.rearrange("p (d t) -> p d t", d=8)
            xt = [xt4[:, d, :] for d in range(8)]
            xT_src = xT.ap().rearrange("(d p) t -> p d t", p=128)

            def load_x_chunk(c, split=False):
                cs = slice(c * 512, (c + 1) * 512)
                if split:
                    # chunk 0 gates the first QK matmul: 8 dma_starts spread
                    # its descriptors across the DMA rings for min latency
                    for d in range(8):
                        nc.sync.dma_start(out=xt4[:, d, cs],
                                          in_=xT_src[:, d, cs])
                else:
                    nc.sync.dma_start(out=xt4[:, :, cs], in_=xT_src[:, :, cs])

            load_x_chunk(0, split=True)
            for c in range(1, 5):
                load_x_chunk(c)
            nc.sync.dma_start(out=wv[:, :], in_=wv_d.ap()[:, :])
            nc.sync.dma_start(out=wo[:, :], in_=wo_d.ap()[:, :])
            for c in range(5, 8):
                load_x_chunk(c)

            wq3 = wq.rearrange("p (t c) -> p t c", c=128)
            wk3 = wk.rearrange("p (t c) -> p t c", c=128)
            wv3 = wv.rearrange("p (t c) -> p t c", c=128)

            # ---- persistent SBUF state ----
            qt = pp.tile([128, T], bf16, tag="qt")
            kt = pp.tile([128, T], bf16, tag="kt")
            vtm = []
            for b in range(B):
                v_sb = pp.tile([128, NH * NKT * VSLOT], bf16, tag=f"v{b}",
                               name=f"v{b}")
                v4 = v_sb.rearrange("p (h k c) -> p h k c", h=NH, k=NKT)
                nc.vector.memset(v4[:, :, :, 0:DH], 0.0)
                nc.vector.memset(v4[:, :, :, 0:1], 1.0)
                vtm.append(v_sb)

            pt_tiles = {}

            def pt_for(ci):
                if ci not in pt_tiles:
                    t = ptp.tile([128, NH * NKT * 512], bf16, tag="pt",
                                 name=f"pt{ci}")
                    pt_tiles[ci] = t.rearrange("p (h k q) -> p h k q",
                                               h=NH, k=NKT)
                return pt_tiles[ci]

            # ---- emission helpers ----
            drain_flip = [0]

            def emit_qk_group(proj_sb, w3, b_sb, c):
                cs = slice(c * 512, (c + 1) * 512)
                ps = psp.tile([128, 512], f32, tag="acc", bufs=2, name="qkps")
                for d in range(8):
                    nc.tensor.matmul(ps[:, :], w3[:, d, :], xt[d][:, cs],
                                     start=(d == 0), stop=(d == 7))
                nc.vector.tensor_scalar_add(proj_sb[:, cs], ps[:, :],
                                            b_sb[:, :])

            def emit_v_group(b, k, slot):
                tok0 = b * S + k * 128
                ps = psp.tile([128, 128], f32, tag=slot,
                              bufs=(2 if slot == "acc" else 1),
                              padded_shape=([128, 1024] if slot == "op"
                                            else [128, 512]),
                              name=f"vps{b}_{k}")
                for d in range(8):
                    nc.tensor.matmul(ps[:, :], xt[d][:, tok0:tok0 + 128],
                                     wv3[:, d, :],
                                     start=(d == 0), stop=(d == 7))
                v4 = vtm[b].rearrange("p (h k c) -> p h k c", h=NH, k=NKT)
                dst = v4[:, :, k, DH:2 * DH]
                src = ps.rearrange("p (h c) -> p h c", h=NH)[:, :, :]
                if drain_flip[0] % 2 == 0:
                    nc.vector.tensor_copy(dst, src)
                else:
                    nc.scalar.activation(dst, src, Copy)
                drain_flip[0] += 1

            def emit_spair(ci, k):
                b, qc = divmod(ci, NQC)
                q0 = b * S + qc * 512
                k0 = b * S + k * 128
                st = psp.tile([128, 1024], f32, tag="st", bufs=2,
                              name=f"st{ci}_{k}")
                for h in range(NH):
                    hp = h * DH
                    nc.tensor.matmul(
                        st[:, h * 512:(h + 1) * 512],
                        kt[hp:hp + DH, k0:k0 + 128],
                        qt[hp:hp + DH, q0:q0 + 512],
                        start=True, stop=True)
                p4 = pt_for(ci)
                if k in SCHR_KTS:
                    out_i16 = p4[:, :, k, :].bitcast(i16)
                    nc.vector.tensor_scalar(
                        out=out_i16, in0=st[:, :], scalar1=A16, scalar2=B16,
                        op0=Mult, op1=Add)
                else:
                    nc.scalar.activation(p4[:, :, k, :], st[:, :], Exp,
                                         scale=SCALE)

            av_tiles = {}

            def emit_av(ci, h, k):
                b = ci // NQC
                v4 = vtm[b].rearrange("p (h k c) -> p h k c", h=NH, k=NKT)
                if (ci, h) not in av_tiles:
                    av_tiles[(ci, h)] = psp.tile(
                        [VSLOT, 512], f32, tag="acc", bufs=2,
                        padded_shape=[128, 512], name=f"av{ci}_{h}")
                av = av_tiles[(ci, h)]
                nc.tensor.matmul(av[0:128, :], v4[:, h, k, :],
                                 pt_for(ci)[:, h, k, :],
                                 start=(k == 0), stop=(k == NKT - 1))

            onorm_tiles = {}

            def emit_norm(ci, h):
                # denom -> reciprocal (DVE) -> broadcast (GpSimd) ->
                # multiply+bf16 (DVE)
                av = av_tiles[(ci, h)]
                recip = rcp.tile([1, 512], f32, tag="recip",
                                 name=f"rc{ci}_{h}")
                nc.vector.reciprocal_approx_fast(out=recip[:, :],
                                                 in_=av[0:1, :])
                rb = rbp.tile([DH, 512], f32, tag="rb", name=f"rb{ci}_{h}")
                nc.gpsimd.partition_broadcast(rb[:, :], recip[:, :])
                if ci not in onorm_tiles:
                    onorm_tiles[ci] = onp.tile([128, 512], bf16, tag="onorm",
                                               name=f"on{ci}")
                onorm = onorm_tiles[ci]
                hp = h * DH
                nc.vector.tensor_tensor(onorm[hp:hp + DH, :],
                                        av[DH:2 * DH, :],
                                        rb[:, :], op=Mult)

            op_tiles = {}

            def _drain_op(ci, s4):
                op = op_tiles[(ci, s4)]
                osb = osp.tile([128, 1024], bf16, tag="outsb",
                               name=f"osb{ci}_{s4}")
                if s4 % 2 == 0:
                    nc.scalar.activation(osb[:, :], op[:, :], Copy)
                else:
                    nc.vector.tensor_copy(osb[:, :], op[:, :])
                b, qc = divmod(ci, NQC)
                r0 = b * S + qc * 512 + s4 * 128
                nc.sync.dma_start(out=outp.ap()[r0:r0 + 128, :],
                                  in_=osb[:, :])

            def emit_outproj_mm(ci, j):
                # j = 0..7 -> s4 = j//2 (128-token block), jc = j%2
                s4, jc = divmod(j, 2)
                onorm = onorm_tiles[ci]
                if (ci, s4) not in op_tiles:
                    op_tiles[(ci, s4)] = psp.tile(
                        [128, 1024], f32, tag="op", bufs=1,
                        name=f"op{ci}_{s4}")
                op = op_tiles[(ci, s4)]
                nc.tensor.matmul(op[:, jc * 512:(jc + 1) * 512],
                                 onorm[:, s4 * 128:(s4 + 1) * 128],
                                 wo[:, jc * 512:(jc + 1) * 512],
                                 start=True, stop=True)
                if jc == 1:
                    _drain_op(ci, s4)

            def emit_outproj_half(ci, j, h):
                # per-head K=64 outproj half (tail only): the h0 half can run
                # before the h1 norm chain completes
                s4, jc = divmod(j, 2)
                onorm = onorm_tiles[ci]
                if (ci, s4) not in op_tiles:
                    # borrow the (now idle) st slots: two tiles in flight
                    op_tiles[(ci, s4)] = psp.tile(
                        [128, 1024], f32, tag="st", bufs=2,
                        name=f"op{ci}_{s4}")
                op = op_tiles[(ci, s4)]
                hp = h * DH
                nc.tensor.matmul(op[:, jc * 512:(jc + 1) * 512],
                                 onorm[hp:hp + DH, s4 * 128:(s4 + 1) * 128],
                                 wo[hp:hp + DH, jc * 512:(jc + 1) * 512],
                                 start=(h == 0), stop=(h == 1))

            # ================= P0: QK projection, batch 0 =================
            for c in range(NQC):
                emit_qk_group(qt, wq3, bq, c)
                emit_qk_group(kt, wk3, bk, c)

            # ===== P1: scores(c0) + QK batch 1 + V both batches =====
            for k in range(NKT):
                emit_spair(0, k)
                if k < 8:
                    c = 4 + k // 2
                    if k % 2 == 0:
                        emit_qk_group(qt, wq3, bq, c)
                    else:
                        emit_qk_group(kt, wk3, bk, c)
                g = 2 * k
                emit_v_group(g // NKT, g % NKT, "acc")
                g = 2 * k + 1
                emit_v_group(g // NKT, g % NKT, "op")

            # ================= attention blocks =================
            # per step: AVs and outproj first, score-pair last — gives the
            # exp engines maximal slack before the PE needs the st slot back
            for ci in range(8):
                for k in range(NKT):
                    h, kk = k // 8, 2 * (k % 8)
                    if k == 8:
                        emit_norm(ci, 0)
                    emit_av(ci, h, kk)
                    emit_av(ci, h, kk + 1)
                    if ci >= 1 and 4 <= k < 12:
                        emit_outproj_mm(ci - 1, k - 4)
                    if ci + 1 < 8:
                        emit_spair(ci + 1, k)
                if ci < 7:
                    emit_norm(ci, 1)

            # tail: normalize the last head per 128-token chunk so each
            # outproj s4 starts as soon as its slice of onorm is ready,
            # instead of waiting for the full-width norm chain
            av7 = av_tiles[(7, 1)]
            recip7 = rcp.tile([1, 512], f32, tag="recip", name="rc7_1")
            nc.vector.reciprocal_approx_fast(out=recip7[:, :],
                                             in_=av7[0:1, :])
            rb7 = rbp.tile([DH, 512], f32, tag="rb", name="rb7_1")
            nc.gpsimd.partition_broadcast(rb7[:, :], recip7[:, :])
            onorm7 = onorm_tiles[7]
            for s4 in range(4):
                qs = slice(s4 * 128, (s4 + 1) * 128)
                nc.vector.tensor_tensor(onorm7[DH:2 * DH, qs],
                                        av7[DH:2 * DH, qs],
                                        rb7[:, qs], op=Mult)
                op_tiles[(7, s4)] = psp.tile([128, 1024], f32, tag="st",
                                             bufs=2, name=f"op7_{s4}")
                op = op_tiles[(7, s4)]
                for jc in range(2):
                    nc.tensor.matmul(op[:, jc * 512:(jc + 1) * 512],
                                     onorm7[:, qs],
                                     wo[:, jc * 512:(jc + 1) * 512],
                                     start=True, stop=True)
                _drain_op(7, s4)

    nc.compile()
    return nc


def _prep_inputs(x, Wq, bq, Wk, bk, Wv, bv, Wo, bo):
    bf16 = ml_dtypes.bfloat16
    xT = np.ascontiguousarray(
        np.asarray(x, dtype=np.float32).reshape(T, D).T).astype(bf16)

    def swz(W, cs):
        # [D, 128] -> SBUF layout [128 p, 8 t, 128 c] flattened
        return np.ascontiguousarray(
            W[:, cs].reshape(8, 128, 128).transpose(1, 0, 2).reshape(128, D)
        ).astype(bf16)

    in_maps = []
    for c in range(NCORES):
        cs = slice(c * 128, (c + 1) * 128)
        in_maps.append({
            "xT": xT,
            "wq": swz(Wq, cs),
            "wk": swz(Wk, cs),
            "wv": swz(Wv, cs),
            "wo": np.ascontiguousarray(Wo[cs, :]).astype(bf16),
            "bq": np.ascontiguousarray(bq[cs]).reshape(128, 1).astype(np.float32),
            "bk": np.ascontiguousarray(bk[cs]).reshape(128, 1).astype(np.float32),
        })
    return in_maps


def kernel(x, Wq, bq, Wk, bk, Wv, bv, Wo, bo, _trace=False, _results=None):
    from concourse.bass_utils import run_bass_kernel_spmd

    x = np.asarray(x); Wq = np.asarray(Wq); Wk = np.asarray(Wk)
    Wv = np.asarray(Wv); Wo = np.asarray(Wo)
    bq = np.asarray(bq); bk = np.asarray(bk); bv = np.asarray(bv)
    bo = np.asarray(bo)

    if "nc" not in _CACHE:
        _CACHE["nc"] = _build_nc()
    nc = _CACHE["nc"]

    in_maps = _prep_inputs(x, Wq, bq, Wk, bk, Wv, bv, Wo, bo)
    res = run_bass_kernel_spmd(
        nc, in_maps, core_ids=list(range(NCORES)), trace=_trace)
    if _results is not None:
        _results.append(res)

    acc = np.zeros((T, D), dtype=np.float32)
    for c in range(NCORES):
        acc += np.asarray(res.results[c]["outp"], dtype=np.float32)
    acc += bv.astype(np.float32) @ Wo.astype(np.float32) + bo.astype(np.float32)
    return acc.reshape(B, S, D)
